# revision 27
# baseline (speedup 1.0000x reference)
"""Sparse avg-pool (segment mean) for Trainium2, 8 NeuronCores — grouped-profile version.

Range-shard coarse ids across cores (core k owns ids [k*31360, (k+1)*31360)),
so no collective is needed.  Each core's 31360 ids form 245 windows of 128
consecutive ids.  The segment-sum runs on the TensorEngine as
onehot^T @ feats accumulated per window in PSUM.

The onehot build is the DVE bottleneck (~163-195ns per tensor_scalar with an
AP scalar, regardless of free size), so the packing amortizes one onehot over
as many matmul slots as possible: a *group* is a set of S slots sharing one
id-profile prof[p] — partition p of every slot in the group holds tokens of
window-relative id prof[p].  The host decomposes each window's id counts into
groups from a size menu ((5,4)/(6,4)/(6,5)... for 2-group windows,
(4,3,1)/(5,3,1)... for 3-group ones), balancing DVE op count against input
DMA bytes (both sit near the wall); half the window positions use the denser
3-group menu, interleaved so every chunk stays locally balanced.  One
`tensor_scalar is_equal` per group (iota row vs per-partition f32 id —
unit-stride bf16 operands, 4x DVE mode) builds the onehot for all S matmuls
of the group.

The group structure is data-dependent and baked into the program at build
time; windows are aligned across cores by heaviness rank and the per-position
sizes unioned, so a single SPMD program serves all 8 cores (a core's cell
assignment stays valid under any elementwise-larger size vector).  Features
are pre-scaled by 1/count on the host (exact int bincount), so PSUM
accumulates the mean directly; the ACT engine copies PSUM out as bf16 to
halve the output DMA, and output DMAs issue from the idle GPSIMD queue so
they can't head-of-line block the next chunk's input DMA on the Sync queue
(that stall was worth ~80us of TensorE idle + pstate resets).
"""
import os
import sys

sys.path.insert(0, "/opt/trn_rl_repo")

import numpy as np

NCORES = 8
C = 64
W = 128          # ids per window
N_WIN = 245      # windows per core
RNG = N_WIN * W  # ids per core (31360)
N_COARSE_PAD = NCORES * RNG
CHUNK_W = 7      # windows per DMA chunk (245 = 35 * 7)

_nc_cache = {}
LAST_RESULT = None


# --------------------------------------------------------------------------
# host-side packing
# --------------------------------------------------------------------------

def _reduce_to_budget(x, l, A, B):
    """Convert A-cells to leftover until sum(x) <= 128, preferring ids where
    the extra A tokens add the fewest B-cells.  Mutates x, l; returns ok."""
    over = int(x.sum()) - 128
    while over > 0:
        cand = np.nonzero(x > 0)[0]
        if cand.size == 0:
            return False
        delta = (-(-(l[cand] + A) // B)) - (-(-l[cand] // B))
        i = int(cand[np.argmin(delta)])
        x[i] -= 1
        l[i] += A
        over -= 1
    return True


def _emit_cells(per_id, S):
    """per_id[i] = token count to place in <=S-token cells of id i."""
    out = []
    for i in np.nonzero(per_id)[0]:
        ci = int(per_id[i])
        while ci > 0:
            m = min(S, ci)
            out.append((int(i), m))
            ci -= m
    return out


def _fit_sizes(c, sizes):
    """Decompose counts c into groups of the given sizes (desc).  Returns
    cells-per-group or None."""
    l = c.astype(np.int64).copy()
    groups = []
    for j, S in enumerate(sizes):
        nxt = sizes[j + 1] if j + 1 < len(sizes) else None
        x = l // S
        rem = l - S * x
        if nxt is None:
            if int((-(-l // S)).sum()) > 128:
                return None
            groups.append(_emit_cells(l, S))
            l = np.zeros_like(l)
        else:
            if not _reduce_to_budget(x, rem, S, nxt):
                return None
            groups.append(_emit_cells(x * S, S))
            l = rem
    if int(l.sum()) != 0:
        return None
    return groups


MENU2 = [(5, 4), (6, 4), (6, 5), (6, 5, 2), (6, 5, 5)]
MENU3 = [(4, 3, 1), (5, 3, 1), (5, 4, 1), (5, 4, 2), (6, 5, 2), (6, 5, 5)]
DENSE_FRACTION = 0.6  # fraction of window positions packed with the 3-group menu


def _cells_5x(c):
    """Fallback: (5,5,...) decomposition, biggest cells first."""
    cells = _emit_cells(c, 5)
    cells.sort(key=lambda t: -t[1])
    groups = [cells[j : j + 128] for j in range(0, len(cells), 128)]
    if not groups:
        groups = [[]]
    sizes = [max((m for _, m in g), default=1) for g in groups]
    return sizes, groups, 99

def _decomp_menu(counts, menu):
    for rank, sizes in enumerate(menu):
        g = _fit_sizes(counts, list(sizes))
        if g is not None:
            return list(sizes), g, rank
    return _cells_5x(counts)


def build_structure(per_core_counts):
    """Per-core decompositions, heaviness-sorted window alignment, union sizes.

    Returns (structure, cells, perm):
      structure[w] = union group sizes at aligned position w
      cells[k][w]  = cell lists for core k's window at position w
      perm[k][w]   = the actual window index of core k at position w
    """
    cells = [[None] * N_WIN for _ in range(NCORES)]
    perm = np.zeros((NCORES, N_WIN), np.int64)
    n_dense = int(N_WIN * DENSE_FRACTION)
    for k in range(NCORES):
        rows = []
        for w in range(N_WIN):
            c = per_core_counts[k][w]
            sizes_k, cells_k, rank = _decomp_menu(c, MENU2)
            rows.append((rank, -int(c.sum()), w, c))
        rows.sort(reverse=True)  # heaviest windows first: they overlap the
        # input stream; the light tail finishes quickly after the last chunk
        for pos, (_, _, w, c) in enumerate(rows):
            # densest positions use the 3-group menu (lower slot count, one
            # extra DVE op); positions are aligned across cores so the
            # menu split must be position-based, not per-core
            dense = int((pos + 1) * DENSE_FRACTION) > int(pos * DENSE_FRACTION)
            menu = MENU3 if dense else MENU2
            sizes_k, cells_k, _ = _decomp_menu(c, menu)
            perm[k][pos] = w
            cells[k][pos] = (sizes_k, cells_k)
    structure = []
    for pos in range(N_WIN):
        union = []
        for k in range(NCORES):
            for j, s in enumerate(cells[k][pos][0]):
                if j < len(union):
                    union[j] = max(union[j], s)
                else:
                    union.append(s)
        if not union:
            union = [1]
        structure.append(union)
    return structure, cells, perm


class Layout:
    """Slot/group offsets derived from the union structure."""

    def __init__(self, structure):
        self.structure = structure
        self.win_slot_off = np.zeros(N_WIN + 1, np.int64)
        self.win_grp_off = np.zeros(N_WIN + 1, np.int64)
        for w, sizes in enumerate(structure):
            self.win_slot_off[w + 1] = self.win_slot_off[w] + sum(sizes)
            self.win_grp_off[w + 1] = self.win_grp_off[w] + len(sizes)
        self.s_tot = int(self.win_slot_off[-1])
        self.g_tot = int(self.win_grp_off[-1])


def shard_inputs(feats, ids, layout, cells, perm):
    """Route rows to owner cores and place tokens per the precomputed cells.

    Builds, per core, index arrays (partition, slot) for every token and does
    one vectorized scatter of the pre-scaled features.
    """
    import ml_dtypes

    ids = np.asarray(ids, dtype=np.int64).ravel()
    feats = np.asarray(feats, dtype=np.float32)

    cnt = np.bincount(ids, minlength=N_COARSE_PAD).astype(np.float32)
    scale = 1.0 / np.maximum(cnt, 1.0)
    feats_scaled = feats * scale[ids][:, None]

    owner = ids // RNG
    order = np.argsort(owner, kind="stable")
    counts_per_core = np.bincount(owner, minlength=NCORES)
    offs = np.zeros(NCORES + 1, np.int64)
    np.cumsum(counts_per_core, out=offs[1:])
    feats_sorted = feats_scaled[order]
    local_sorted = (ids - owner * RNG)[order]

    iota = np.broadcast_to(
        np.arange(W, dtype=np.float32), (128, W)
    ).astype(ml_dtypes.bfloat16)

    in_maps = []
    for k in range(NCORES):
        fk = feats_sorted[offs[k] : offs[k + 1]]
        lk = local_sorted[offs[k] : offs[k + 1]]
        fa = np.zeros((128, layout.s_tot, C), np.float32)
        prof = np.zeros((128, layout.g_tot), np.float32)
        if lk.shape[0]:
            sorder = np.argsort(lk, kind="stable")
            ls = lk[sorder]
            fs = fk[sorder]
            win = ls >> 7
            wstart = np.searchsorted(win, np.arange(N_WIN + 1))
            dst_p = np.empty(ls.shape[0], np.int64)
            dst_s = np.empty(ls.shape[0], np.int64)
            for pos in range(N_WIN):
                w = int(perm[k][pos])
                lo, hi = int(wstart[w]), int(wstart[w + 1])
                if lo == hi:
                    continue
                wrel = (ls[lo:hi] & 127).astype(np.int64)
                worder = np.argsort(wrel, kind="stable")
                counts = np.bincount(wrel, minlength=W)
                id_start = np.zeros(W + 1, np.int64)
                np.cumsum(counts, out=id_start[1:])
                used = np.zeros(W, np.int64)
                slot0 = int(layout.win_slot_off[pos])
                grp0 = int(layout.win_grp_off[pos])
                s_off = 0
                gcells_list = cells[k][pos][1]
                for g, S in enumerate(layout.structure[pos]):
                    gcells = gcells_list[g] if g < len(gcells_list) else []
                    for p, (i, m) in enumerate(gcells):
                        prof[p, grp0 + g] = i
                        u = used[i]
                        toks = worder[id_start[i] + u : id_start[i] + u + m]
                        used[i] = u + m
                        dst_p[lo + toks] = p
                        dst_s[lo + toks] = slot0 + s_off + np.arange(m)
                    s_off += S
            fa[dst_p, dst_s, :] = fs
        in_maps.append(
            {
                "feats": fa.astype(ml_dtypes.bfloat16),
                "ids": prof,
                "iota": iota,
            }
        )
    return in_maps


# --------------------------------------------------------------------------
# device program
# --------------------------------------------------------------------------

def build_nc(layout):
    from concourse import bacc, mybir, tile

    bf16 = mybir.dt.bfloat16
    f32 = mybir.dt.float32
    nc = bacc.Bacc("TRN2", target_bir_lowering=False)
    feats_ext = nc.declare_dram_parameter(
        "feats", [128, layout.s_tot, C], bf16, isOutput=False
    )
    ids_ext = nc.declare_dram_parameter("ids", [128, layout.g_tot], f32, isOutput=False)
    iota_ext = nc.declare_dram_parameter("iota", [128, W], bf16, isOutput=False)
    n_chunks = N_WIN // CHUNK_W
    out_ext = nc.declare_dram_parameter("out", [128, N_WIN, C], bf16, isOutput=True)

    # chunk slot extents
    chunk_lo = [int(layout.win_slot_off[ch * CHUNK_W]) for ch in range(n_chunks)]
    chunk_hi = [int(layout.win_slot_off[(ch + 1) * CHUNK_W]) for ch in range(n_chunks)]
    max_chunk_slots = max(hi - lo for lo, hi in zip(chunk_lo, chunk_hi))

    with tile.TileContext(nc) as tc:
        with (
            tc.tile_pool(name="stage", bufs=3) as stagep,
            tc.tile_pool(name="oh", bufs=10) as ohp,
            tc.tile_pool(name="psum", bufs=8, space="PSUM") as psump,
            tc.tile_pool(name="fin", bufs=3) as finp,
            tc.tile_pool(name="cst", bufs=1) as cstp,
        ):
            iota_t = cstp.tile([128, W], bf16)
            nc.scalar.dma_start(out=iota_t[:], in_=iota_ext[:])
            ids_t = cstp.tile([128, layout.g_tot], f32)
            nc.scalar.dma_start(out=ids_t[:], in_=ids_ext[:])
            ot_all = cstp.tile([128, N_WIN, C], bf16)

            for ch in range(n_chunks):
                lo, hi = chunk_lo[ch], chunk_hi[ch]
                src = stagep.tile([128, max_chunk_slots, C], bf16, tag="src")
                nc.sync.dma_start(
                    out=src[:, : hi - lo, :], in_=feats_ext[:, lo:hi, :]
                )
                for wi in range(CHUNK_W):
                    w = ch * CHUNK_W + wi
                    sizes = layout.structure[w]
                    ps = psump.tile([128, C], f32, tag="ps")
                    n_slots = sum(sizes)
                    s_base = int(layout.win_slot_off[w]) - lo
                    g_base = int(layout.win_grp_off[w])
                    s_off = 0
                    for g, S in enumerate(sizes):
                        oh = ohp.tile([128, W], bf16, tag="oh")
                        nc.vector.tensor_scalar(
                            out=oh[:],
                            in0=iota_t[:],
                            scalar1=ids_t[:, g_base + g : g_base + g + 1],
                            scalar2=None,
                            op0=mybir.AluOpType.is_equal,
                        )
                        for s in range(S):
                            slot = s_base + s_off + s
                            nc.tensor.matmul(
                                out=ps[:],
                                lhsT=oh[:],
                                rhs=src[:, slot, :],
                                start=(s_off + s == 0),
                                stop=(s_off + s == n_slots - 1),
                            )
                        s_off += S
                    nc.scalar.activation(
                        ot_all[:, w, :], ps[:], mybir.ActivationFunctionType.Copy
                    )
                # output DMA in 5-chunk super-bursts on the idle Pool queue so
                # it can't head-of-line block input DMAs and transfers at full
                # rate (574KB vs 114KB bursts)
                if ch % 5 == 4 or ch == n_chunks - 1:
                    p0 = (ch - ch % 5) * CHUNK_W
                    p1 = (ch + 1) * CHUNK_W
                    nc.gpsimd.dma_start(
                        out=out_ext[:, p0:p1, :], in_=ot_all[:, p0:p1, :]
                    )
    nc.compile()
    return nc


def assemble_output(results, n_coarse, perm):
    out = np.empty((NCORES * RNG, C), np.float32)
    for k in range(NCORES):
        r = np.asarray(results[k]["out"], dtype=np.float32)  # [128, N_WIN, C]
        by_pos = r.transpose(1, 0, 2)
        core = out[k * RNG : (k + 1) * RNG].reshape(N_WIN, W, C)
        core[perm[k]] = by_pos
    return out[:n_coarse]


def _install_axon_hooks_shim():
    """Provide antenv.axon_hooks + the ctypes NTFF hook if the image lacks it."""
    import contextlib
    import ctypes
    import types

    try:
        from antenv.axon_hooks import get_axon_ntff_profile_hook  # noqa: F401

        return
    except ImportError:
        pass
    import antenv

    mod = types.ModuleType("antenv.axon_hooks")
    state = {"h": None}
    mod.set_axon_ntff_profile_hook = lambda h: state.__setitem__("h", h)
    mod.get_axon_ntff_profile_hook = lambda: state["h"]
    antenv.axon_hooks = mod
    sys.modules["antenv.axon_hooks"] = mod

    so_path = "/opt/axon/libaxon_pjrt.so"
    if not os.path.exists(so_path):
        return
    lib = ctypes.CDLL(so_path)
    if not hasattr(lib, "axon_start_nrt_profile"):
        return
    lib.axon_start_nrt_profile.argtypes = [
        ctypes.POINTER(ctypes.c_int64),
        ctypes.c_size_t,
    ]
    lib.axon_start_nrt_profile.restype = ctypes.c_int64
    lib.axon_stop_nrt_profile.argtypes = [ctypes.c_char_p]
    lib.axon_stop_nrt_profile.restype = ctypes.c_int64

    @contextlib.contextmanager
    def _hook(output_dir, device_ids):
        import jax

        jax.devices()
        if device_ids:
            idsv = (ctypes.c_int64 * len(device_ids))(*device_ids)
            rc = lib.axon_start_nrt_profile(idsv, len(device_ids))
        else:
            rc = lib.axon_start_nrt_profile(None, 0)
        if rc != 0:
            raise RuntimeError(f"axon_start_nrt_profile rc={rc}")
        try:
            yield
        finally:
            nfiles = lib.axon_stop_nrt_profile(str(output_dir).encode())
            print(f"profile: {nfiles} file(s) written to {output_dir}", file=sys.stderr)

    state["h"] = _hook


def _per_core_counts(coarse_ids):
    """[NCORES][N_WIN][W] token counts."""
    ids = np.asarray(coarse_ids, dtype=np.int64).ravel()
    cnt = np.bincount(ids, minlength=N_COARSE_PAD)
    return cnt.reshape(NCORES, N_WIN, W)


def kernel(fine_feats, coarse_ids, num_coarse):
    global LAST_RESULT
    from concourse.bass_utils import run_bass_kernel_spmd

    counts = _per_core_counts(coarse_ids)
    structure, cells, perm = build_structure(counts)
    layout = Layout(structure)
    in_maps = shard_inputs(fine_feats, coarse_ids, layout, cells, perm)
    key = tuple(tuple(s) for s in structure)
    if key not in _nc_cache:
        _nc_cache.clear()
        _nc_cache[key] = build_nc(layout)
    nc = _nc_cache[key]
    trace = bool(int(os.environ.get("KERNEL_TRACE", "0")))
    if trace:
        _install_axon_hooks_shim()
    res = run_bass_kernel_spmd(nc, in_maps, core_ids=list(range(NCORES)), trace=trace)
    LAST_RESULT = res
    return assemble_output(res.results, int(num_coarse), perm)


# revision 28
# speedup vs baseline: 1.0030x; 1.0030x over previous
"""Sparse avg-pool (segment mean) for Trainium2, 8 NeuronCores — grouped-profile version.

Range-shard coarse ids across cores (core k owns ids [k*31360, (k+1)*31360)),
so no collective is needed.  Each core's 31360 ids form 245 windows of 128
consecutive ids.  The segment-sum runs on the TensorEngine as
onehot^T @ feats accumulated per window in PSUM.

The onehot build is the DVE bottleneck (~163-195ns per tensor_scalar with an
AP scalar, regardless of free size), so the packing amortizes one onehot over
as many matmul slots as possible: a *group* is a set of S slots sharing one
id-profile prof[p] — partition p of every slot in the group holds tokens of
window-relative id prof[p].  The host decomposes each window's id counts into
groups from a size menu ((5,4)/(6,4)/(6,5)... for 2-group windows,
(4,3,1)/(5,3,1)... for 3-group ones), balancing DVE op count against input
DMA bytes (both sit near the wall); half the window positions use the denser
3-group menu, interleaved so every chunk stays locally balanced.  One
`tensor_scalar is_equal` per group (iota row vs per-partition f32 id —
unit-stride bf16 operands, 4x DVE mode) builds the onehot for all S matmuls
of the group.

The group structure is data-dependent and baked into the program at build
time; windows are aligned across cores by heaviness rank and the per-position
sizes unioned, so a single SPMD program serves all 8 cores (a core's cell
assignment stays valid under any elementwise-larger size vector).  Features
are pre-scaled by 1/count on the host (exact int bincount), so PSUM
accumulates the mean directly; the ACT engine copies PSUM out as bf16 to
halve the output DMA, and output DMAs issue from the idle GPSIMD queue so
they can't head-of-line block the next chunk's input DMA on the Sync queue
(that stall was worth ~80us of TensorE idle + pstate resets).
"""
import os
import sys

sys.path.insert(0, "/opt/trn_rl_repo")

import numpy as np

NCORES = 8
C = 64
W = 128          # ids per window
N_WIN = 245      # windows per core
RNG = N_WIN * W  # ids per core (31360)
N_COARSE_PAD = NCORES * RNG
CHUNK_W = 7      # windows per DMA chunk (245 = 35 * 7)

_nc_cache = {}
LAST_RESULT = None


# --------------------------------------------------------------------------
# host-side packing
# --------------------------------------------------------------------------

def _reduce_to_budget(x, l, A, B):
    """Convert A-cells to leftover until sum(x) <= 128, preferring ids where
    the extra A tokens add the fewest B-cells.  Mutates x, l; returns ok."""
    over = int(x.sum()) - 128
    while over > 0:
        cand = np.nonzero(x > 0)[0]
        if cand.size == 0:
            return False
        delta = (-(-(l[cand] + A) // B)) - (-(-l[cand] // B))
        i = int(cand[np.argmin(delta)])
        x[i] -= 1
        l[i] += A
        over -= 1
    return True


def _emit_cells(per_id, S):
    """per_id[i] = token count to place in <=S-token cells of id i."""
    out = []
    for i in np.nonzero(per_id)[0]:
        ci = int(per_id[i])
        while ci > 0:
            m = min(S, ci)
            out.append((int(i), m))
            ci -= m
    return out


def _fit_sizes(c, sizes):
    """Decompose counts c into groups of the given sizes (desc).  Returns
    cells-per-group or None."""
    l = c.astype(np.int64).copy()
    groups = []
    for j, S in enumerate(sizes):
        nxt = sizes[j + 1] if j + 1 < len(sizes) else None
        x = l // S
        rem = l - S * x
        if nxt is None:
            if int((-(-l // S)).sum()) > 128:
                return None
            groups.append(_emit_cells(l, S))
            l = np.zeros_like(l)
        else:
            if not _reduce_to_budget(x, rem, S, nxt):
                return None
            groups.append(_emit_cells(x * S, S))
            l = rem
    if int(l.sum()) != 0:
        return None
    return groups


MENU2 = [(5, 4), (6, 4), (6, 5), (6, 5, 2), (6, 5, 5)]
MENU3 = [(4, 3, 1), (5, 3, 1), (5, 4, 1), (5, 4, 2), (6, 5, 2), (6, 5, 5)]
DENSE_FRACTION = 0.5  # fraction of window positions packed with the 3-group menu


def _cells_5x(c):
    """Fallback: (5,5,...) decomposition, biggest cells first."""
    cells = _emit_cells(c, 5)
    cells.sort(key=lambda t: -t[1])
    groups = [cells[j : j + 128] for j in range(0, len(cells), 128)]
    if not groups:
        groups = [[]]
    sizes = [max((m for _, m in g), default=1) for g in groups]
    return sizes, groups, 99

def _decomp_menu(counts, menu):
    for rank, sizes in enumerate(menu):
        g = _fit_sizes(counts, list(sizes))
        if g is not None:
            return list(sizes), g, rank
    return _cells_5x(counts)


def build_structure(per_core_counts):
    """Per-core decompositions, heaviness-sorted window alignment, union sizes.

    Returns (structure, cells, perm):
      structure[w] = union group sizes at aligned position w
      cells[k][w]  = cell lists for core k's window at position w
      perm[k][w]   = the actual window index of core k at position w
    """
    cells = [[None] * N_WIN for _ in range(NCORES)]
    perm = np.zeros((NCORES, N_WIN), np.int64)
    n_dense = int(N_WIN * DENSE_FRACTION)
    for k in range(NCORES):
        rows = []
        for w in range(N_WIN):
            c = per_core_counts[k][w]
            sizes_k, cells_k, rank = _decomp_menu(c, MENU2)
            rows.append((rank, -int(c.sum()), w, c))
        rows.sort()
        for pos, (_, _, w, c) in enumerate(rows):
            # densest positions use the 3-group menu (lower slot count, one
            # extra DVE op); positions are aligned across cores so the
            # menu split must be position-based, not per-core
            dense = int((pos + 1) * DENSE_FRACTION) > int(pos * DENSE_FRACTION)
            menu = MENU3 if dense else MENU2
            sizes_k, cells_k, _ = _decomp_menu(c, menu)
            perm[k][pos] = w
            cells[k][pos] = (sizes_k, cells_k)
    structure = []
    for pos in range(N_WIN):
        union = []
        for k in range(NCORES):
            for j, s in enumerate(cells[k][pos][0]):
                if j < len(union):
                    union[j] = max(union[j], s)
                else:
                    union.append(s)
        if not union:
            union = [1]
        structure.append(union)
    return structure, cells, perm


class Layout:
    """Slot/group offsets derived from the union structure."""

    def __init__(self, structure):
        self.structure = structure
        self.win_slot_off = np.zeros(N_WIN + 1, np.int64)
        self.win_grp_off = np.zeros(N_WIN + 1, np.int64)
        for w, sizes in enumerate(structure):
            self.win_slot_off[w + 1] = self.win_slot_off[w] + sum(sizes)
            self.win_grp_off[w + 1] = self.win_grp_off[w] + len(sizes)
        self.s_tot = int(self.win_slot_off[-1])
        self.g_tot = int(self.win_grp_off[-1])


def shard_inputs(feats, ids, layout, cells, perm):
    """Route rows to owner cores and place tokens per the precomputed cells.

    Builds, per core, index arrays (partition, slot) for every token and does
    one vectorized scatter of the pre-scaled features.
    """
    import ml_dtypes

    ids = np.asarray(ids, dtype=np.int64).ravel()
    feats = np.asarray(feats, dtype=np.float32)

    cnt = np.bincount(ids, minlength=N_COARSE_PAD).astype(np.float32)
    scale = 1.0 / np.maximum(cnt, 1.0)
    feats_scaled = feats * scale[ids][:, None]

    owner = ids // RNG
    order = np.argsort(owner, kind="stable")
    counts_per_core = np.bincount(owner, minlength=NCORES)
    offs = np.zeros(NCORES + 1, np.int64)
    np.cumsum(counts_per_core, out=offs[1:])
    feats_sorted = feats_scaled[order]
    local_sorted = (ids - owner * RNG)[order]

    iota = np.broadcast_to(
        np.arange(W, dtype=np.float32), (128, W)
    ).astype(ml_dtypes.bfloat16)

    in_maps = []
    for k in range(NCORES):
        fk = feats_sorted[offs[k] : offs[k + 1]]
        lk = local_sorted[offs[k] : offs[k + 1]]
        fa = np.zeros((128, layout.s_tot, C), np.float32)
        prof = np.zeros((128, layout.g_tot), np.float32)
        if lk.shape[0]:
            sorder = np.argsort(lk, kind="stable")
            ls = lk[sorder]
            fs = fk[sorder]
            win = ls >> 7
            wstart = np.searchsorted(win, np.arange(N_WIN + 1))
            dst_p = np.empty(ls.shape[0], np.int64)
            dst_s = np.empty(ls.shape[0], np.int64)
            for pos in range(N_WIN):
                w = int(perm[k][pos])
                lo, hi = int(wstart[w]), int(wstart[w + 1])
                if lo == hi:
                    continue
                wrel = (ls[lo:hi] & 127).astype(np.int64)
                worder = np.argsort(wrel, kind="stable")
                counts = np.bincount(wrel, minlength=W)
                id_start = np.zeros(W + 1, np.int64)
                np.cumsum(counts, out=id_start[1:])
                used = np.zeros(W, np.int64)
                slot0 = int(layout.win_slot_off[pos])
                grp0 = int(layout.win_grp_off[pos])
                s_off = 0
                gcells_list = cells[k][pos][1]
                for g, S in enumerate(layout.structure[pos]):
                    gcells = gcells_list[g] if g < len(gcells_list) else []
                    for p, (i, m) in enumerate(gcells):
                        prof[p, grp0 + g] = i
                        u = used[i]
                        toks = worder[id_start[i] + u : id_start[i] + u + m]
                        used[i] = u + m
                        dst_p[lo + toks] = p
                        dst_s[lo + toks] = slot0 + s_off + np.arange(m)
                    s_off += S
            fa[dst_p, dst_s, :] = fs
        in_maps.append(
            {
                "feats": fa.astype(ml_dtypes.bfloat16),
                "ids": prof,
                "iota": iota,
            }
        )
    return in_maps


# --------------------------------------------------------------------------
# device program
# --------------------------------------------------------------------------

def build_nc(layout):
    from concourse import bacc, mybir, tile

    bf16 = mybir.dt.bfloat16
    f32 = mybir.dt.float32
    nc = bacc.Bacc("TRN2", target_bir_lowering=False)
    feats_ext = nc.declare_dram_parameter(
        "feats", [128, layout.s_tot, C], bf16, isOutput=False
    )
    ids_ext = nc.declare_dram_parameter("ids", [128, layout.g_tot], f32, isOutput=False)
    iota_ext = nc.declare_dram_parameter("iota", [128, W], bf16, isOutput=False)
    n_chunks = N_WIN // CHUNK_W
    out_ext = nc.declare_dram_parameter(
        "out", [n_chunks, 128, CHUNK_W, C], bf16, isOutput=True
    )

    # chunk slot extents
    chunk_lo = [int(layout.win_slot_off[ch * CHUNK_W]) for ch in range(n_chunks)]
    chunk_hi = [int(layout.win_slot_off[(ch + 1) * CHUNK_W]) for ch in range(n_chunks)]
    max_chunk_slots = max(hi - lo for lo, hi in zip(chunk_lo, chunk_hi))

    with tile.TileContext(nc) as tc:
        with (
            tc.tile_pool(name="stage", bufs=3) as stagep,
            tc.tile_pool(name="oh", bufs=10) as ohp,
            tc.tile_pool(name="psum", bufs=8, space="PSUM") as psump,
            tc.tile_pool(name="fin", bufs=3) as finp,
            tc.tile_pool(name="cst", bufs=1) as cstp,
        ):
            iota_t = cstp.tile([128, W], bf16)
            nc.sync.dma_start(out=iota_t[:], in_=iota_ext[:])
            ids_t = cstp.tile([128, layout.g_tot], f32)
            nc.sync.dma_start(out=ids_t[:], in_=ids_ext[:])

            for ch in range(n_chunks):
                lo, hi = chunk_lo[ch], chunk_hi[ch]
                src = stagep.tile([128, max_chunk_slots, C], bf16, tag="src")
                nc.sync.dma_start(
                    out=src[:, : hi - lo, :], in_=feats_ext[:, lo:hi, :]
                )
                ot = finp.tile([128, CHUNK_W, C], bf16, tag="ot")
                for wi in range(CHUNK_W):
                    w = ch * CHUNK_W + wi
                    sizes = layout.structure[w]
                    ps = psump.tile([128, C], f32, tag="ps")
                    n_slots = sum(sizes)
                    s_base = int(layout.win_slot_off[w]) - lo
                    g_base = int(layout.win_grp_off[w])
                    s_off = 0
                    for g, S in enumerate(sizes):
                        oh = ohp.tile([128, W], bf16, tag="oh")
                        nc.vector.tensor_scalar(
                            out=oh[:],
                            in0=iota_t[:],
                            scalar1=ids_t[:, g_base + g : g_base + g + 1],
                            scalar2=None,
                            op0=mybir.AluOpType.is_equal,
                        )
                        for s in range(S):
                            slot = s_base + s_off + s
                            nc.tensor.matmul(
                                out=ps[:],
                                lhsT=oh[:],
                                rhs=src[:, slot, :],
                                start=(s_off + s == 0),
                                stop=(s_off + s == n_slots - 1),
                            )
                        s_off += S
                    nc.scalar.activation(
                        ot[:, wi, :], ps[:], mybir.ActivationFunctionType.Copy
                    )
                # output DMA on the idle Pool queue so it can't head-of-line
                # block the next chunk's input DMA on the Sync queue
                nc.gpsimd.dma_start(out=out_ext[ch], in_=ot[:])
    nc.compile()
    return nc


def assemble_output(results, n_coarse, perm):
    out = np.empty((NCORES * RNG, C), np.float32)
    for k in range(NCORES):
        r = np.asarray(results[k]["out"], dtype=np.float32)  # [n_chunks,128,CHUNK_W,C]
        by_pos = r.transpose(0, 2, 1, 3).reshape(N_WIN, W, C)
        core = out[k * RNG : (k + 1) * RNG].reshape(N_WIN, W, C)
        core[perm[k]] = by_pos
    return out[:n_coarse]


def _install_axon_hooks_shim():
    """Provide antenv.axon_hooks + the ctypes NTFF hook if the image lacks it."""
    import contextlib
    import ctypes
    import types

    try:
        from antenv.axon_hooks import get_axon_ntff_profile_hook  # noqa: F401

        return
    except ImportError:
        pass
    import antenv

    mod = types.ModuleType("antenv.axon_hooks")
    state = {"h": None}
    mod.set_axon_ntff_profile_hook = lambda h: state.__setitem__("h", h)
    mod.get_axon_ntff_profile_hook = lambda: state["h"]
    antenv.axon_hooks = mod
    sys.modules["antenv.axon_hooks"] = mod

    so_path = "/opt/axon/libaxon_pjrt.so"
    if not os.path.exists(so_path):
        return
    lib = ctypes.CDLL(so_path)
    if not hasattr(lib, "axon_start_nrt_profile"):
        return
    lib.axon_start_nrt_profile.argtypes = [
        ctypes.POINTER(ctypes.c_int64),
        ctypes.c_size_t,
    ]
    lib.axon_start_nrt_profile.restype = ctypes.c_int64
    lib.axon_stop_nrt_profile.argtypes = [ctypes.c_char_p]
    lib.axon_stop_nrt_profile.restype = ctypes.c_int64

    @contextlib.contextmanager
    def _hook(output_dir, device_ids):
        import jax

        jax.devices()
        if device_ids:
            idsv = (ctypes.c_int64 * len(device_ids))(*device_ids)
            rc = lib.axon_start_nrt_profile(idsv, len(device_ids))
        else:
            rc = lib.axon_start_nrt_profile(None, 0)
        if rc != 0:
            raise RuntimeError(f"axon_start_nrt_profile rc={rc}")
        try:
            yield
        finally:
            nfiles = lib.axon_stop_nrt_profile(str(output_dir).encode())
            print(f"profile: {nfiles} file(s) written to {output_dir}", file=sys.stderr)

    state["h"] = _hook


def _per_core_counts(coarse_ids):
    """[NCORES][N_WIN][W] token counts."""
    ids = np.asarray(coarse_ids, dtype=np.int64).ravel()
    cnt = np.bincount(ids, minlength=N_COARSE_PAD)
    return cnt.reshape(NCORES, N_WIN, W)


def kernel(fine_feats, coarse_ids, num_coarse):
    global LAST_RESULT
    from concourse.bass_utils import run_bass_kernel_spmd

    counts = _per_core_counts(coarse_ids)
    structure, cells, perm = build_structure(counts)
    layout = Layout(structure)
    in_maps = shard_inputs(fine_feats, coarse_ids, layout, cells, perm)
    key = tuple(tuple(s) for s in structure)
    if key not in _nc_cache:
        _nc_cache.clear()
        _nc_cache[key] = build_nc(layout)
    nc = _nc_cache[key]
    trace = bool(int(os.environ.get("KERNEL_TRACE", "0")))
    if trace:
        _install_axon_hooks_shim()
    res = run_bass_kernel_spmd(nc, in_maps, core_ids=list(range(NCORES)), trace=trace)
    LAST_RESULT = res
    return assemble_output(res.results, int(num_coarse), perm)


# revision 29
# speedup vs baseline: 1.0101x; 1.0071x over previous
"""Sparse avg-pool (segment mean) for Trainium2, 8 NeuronCores — grouped-profile version.

Range-shard coarse ids across cores (core k owns ids [k*31360, (k+1)*31360)),
so no collective is needed.  Each core's 31360 ids form 245 windows of 128
consecutive ids.  The segment-sum runs on the TensorEngine as
onehot^T @ feats accumulated per window in PSUM.

The onehot build is the DVE bottleneck (~163-195ns per tensor_scalar with an
AP scalar, regardless of free size), so the packing amortizes one onehot over
as many matmul slots as possible: a *group* is a set of S slots sharing one
id-profile prof[p] — partition p of every slot in the group holds tokens of
window-relative id prof[p].  The host decomposes each window's id counts into
groups from a size menu ((5,4)/(6,4)/(6,5)... for 2-group windows,
(4,3,1)/(5,3,1)... for 3-group ones), balancing DVE op count against input
DMA bytes (both sit near the wall); half the window positions use the denser
3-group menu, interleaved so every chunk stays locally balanced.  One
`tensor_scalar is_equal` per group (iota row vs per-partition f32 id —
unit-stride bf16 operands, 4x DVE mode) builds the onehot for all S matmuls
of the group.

The group structure is data-dependent and baked into the program at build
time; windows are aligned across cores by heaviness rank and the per-position
sizes unioned, so a single SPMD program serves all 8 cores (a core's cell
assignment stays valid under any elementwise-larger size vector).  Features
are pre-scaled by 1/count on the host (exact int bincount), so PSUM
accumulates the mean directly; the ACT engine copies PSUM out as bf16 to
halve the output DMA, and output DMAs issue from the idle GPSIMD queue so
they can't head-of-line block the next chunk's input DMA on the Sync queue
(that stall was worth ~80us of TensorE idle + pstate resets).
"""
import os
import sys

sys.path.insert(0, "/opt/trn_rl_repo")

import numpy as np

NCORES = 8
C = 64
W = 128          # ids per window
N_WIN = 245      # windows per core
RNG = N_WIN * W  # ids per core (31360)
N_COARSE_PAD = NCORES * RNG
CHUNK_W = 7      # windows per DMA chunk (245 = 35 * 7)

_nc_cache = {}
LAST_RESULT = None


# --------------------------------------------------------------------------
# host-side packing
# --------------------------------------------------------------------------

def _reduce_to_budget(x, l, A, B):
    """Convert A-cells to leftover until sum(x) <= 128, preferring ids where
    the extra A tokens add the fewest B-cells.  Mutates x, l; returns ok."""
    over = int(x.sum()) - 128
    while over > 0:
        cand = np.nonzero(x > 0)[0]
        if cand.size == 0:
            return False
        delta = (-(-(l[cand] + A) // B)) - (-(-l[cand] // B))
        i = int(cand[np.argmin(delta)])
        x[i] -= 1
        l[i] += A
        over -= 1
    return True


def _emit_cells(per_id, S):
    """per_id[i] = token count to place in <=S-token cells of id i."""
    out = []
    for i in np.nonzero(per_id)[0]:
        ci = int(per_id[i])
        while ci > 0:
            m = min(S, ci)
            out.append((int(i), m))
            ci -= m
    return out


def _fit_sizes(c, sizes):
    """Decompose counts c into groups of the given sizes (desc).  Returns
    cells-per-group or None."""
    l = c.astype(np.int64).copy()
    groups = []
    for j, S in enumerate(sizes):
        nxt = sizes[j + 1] if j + 1 < len(sizes) else None
        x = l // S
        rem = l - S * x
        if nxt is None:
            if int((-(-l // S)).sum()) > 128:
                return None
            groups.append(_emit_cells(l, S))
            l = np.zeros_like(l)
        else:
            if not _reduce_to_budget(x, rem, S, nxt):
                return None
            groups.append(_emit_cells(x * S, S))
            l = rem
    if int(l.sum()) != 0:
        return None
    return groups


MENU2 = [(5, 4), (6, 4), (6, 5), (6, 5, 2), (6, 5, 5)]
MENU3 = [(4, 3, 1), (5, 3, 1), (5, 4, 1), (5, 4, 2), (6, 5, 2), (6, 5, 5)]
DENSE_FRACTION = 0.6  # fraction of window positions packed with the 3-group menu


def _cells_5x(c):
    """Fallback: (5,5,...) decomposition, biggest cells first."""
    cells = _emit_cells(c, 5)
    cells.sort(key=lambda t: -t[1])
    groups = [cells[j : j + 128] for j in range(0, len(cells), 128)]
    if not groups:
        groups = [[]]
    sizes = [max((m for _, m in g), default=1) for g in groups]
    return sizes, groups, 99

def _decomp_menu(counts, menu):
    for rank, sizes in enumerate(menu):
        g = _fit_sizes(counts, list(sizes))
        if g is not None:
            return list(sizes), g, rank
    return _cells_5x(counts)


def build_structure(per_core_counts):
    """Per-core decompositions, heaviness-sorted window alignment, union sizes.

    Returns (structure, cells, perm):
      structure[w] = union group sizes at aligned position w
      cells[k][w]  = cell lists for core k's window at position w
      perm[k][w]   = the actual window index of core k at position w
    """
    cells = [[None] * N_WIN for _ in range(NCORES)]
    perm = np.zeros((NCORES, N_WIN), np.int64)
    n_dense = int(N_WIN * DENSE_FRACTION)
    for k in range(NCORES):
        rows = []
        for w in range(N_WIN):
            c = per_core_counts[k][w]
            sizes_k, cells_k, rank = _decomp_menu(c, MENU2)
            rows.append((rank, -int(c.sum()), w, c))
        rows.sort()
        for pos, (_, _, w, c) in enumerate(rows):
            # densest positions use the 3-group menu (lower slot count, one
            # extra DVE op); positions are aligned across cores so the
            # menu split must be position-based, not per-core
            dense = int((pos + 1) * DENSE_FRACTION) > int(pos * DENSE_FRACTION)
            menu = MENU3 if dense else MENU2
            sizes_k, cells_k, _ = _decomp_menu(c, menu)
            perm[k][pos] = w
            cells[k][pos] = (sizes_k, cells_k)
    structure = []
    for pos in range(N_WIN):
        union = []
        for k in range(NCORES):
            for j, s in enumerate(cells[k][pos][0]):
                if j < len(union):
                    union[j] = max(union[j], s)
                else:
                    union.append(s)
        if not union:
            union = [1]
        structure.append(union)
    return structure, cells, perm


class Layout:
    """Slot/group offsets derived from the union structure."""

    def __init__(self, structure):
        self.structure = structure
        self.win_slot_off = np.zeros(N_WIN + 1, np.int64)
        self.win_grp_off = np.zeros(N_WIN + 1, np.int64)
        for w, sizes in enumerate(structure):
            self.win_slot_off[w + 1] = self.win_slot_off[w] + sum(sizes)
            self.win_grp_off[w + 1] = self.win_grp_off[w] + len(sizes)
        self.s_tot = int(self.win_slot_off[-1])
        self.g_tot = int(self.win_grp_off[-1])


def shard_inputs(feats, ids, layout, cells, perm):
    """Route rows to owner cores and place tokens per the precomputed cells.

    Builds, per core, index arrays (partition, slot) for every token and does
    one vectorized scatter of the pre-scaled features.
    """
    import ml_dtypes

    ids = np.asarray(ids, dtype=np.int64).ravel()
    feats = np.asarray(feats, dtype=np.float32)

    cnt = np.bincount(ids, minlength=N_COARSE_PAD).astype(np.float32)
    scale = 1.0 / np.maximum(cnt, 1.0)
    feats_scaled = feats * scale[ids][:, None]

    owner = ids // RNG
    order = np.argsort(owner, kind="stable")
    counts_per_core = np.bincount(owner, minlength=NCORES)
    offs = np.zeros(NCORES + 1, np.int64)
    np.cumsum(counts_per_core, out=offs[1:])
    feats_sorted = feats_scaled[order]
    local_sorted = (ids - owner * RNG)[order]

    iota = np.broadcast_to(
        np.arange(W, dtype=np.float32), (128, W)
    ).astype(ml_dtypes.bfloat16)

    in_maps = []
    for k in range(NCORES):
        fk = feats_sorted[offs[k] : offs[k + 1]]
        lk = local_sorted[offs[k] : offs[k + 1]]
        fa = np.zeros((128, layout.s_tot, C), np.float32)
        prof = np.zeros((128, layout.g_tot), np.float32)
        if lk.shape[0]:
            sorder = np.argsort(lk, kind="stable")
            ls = lk[sorder]
            fs = fk[sorder]
            win = ls >> 7
            wstart = np.searchsorted(win, np.arange(N_WIN + 1))
            dst_p = np.empty(ls.shape[0], np.int64)
            dst_s = np.empty(ls.shape[0], np.int64)
            for pos in range(N_WIN):
                w = int(perm[k][pos])
                lo, hi = int(wstart[w]), int(wstart[w + 1])
                if lo == hi:
                    continue
                wrel = (ls[lo:hi] & 127).astype(np.int64)
                worder = np.argsort(wrel, kind="stable")
                counts = np.bincount(wrel, minlength=W)
                id_start = np.zeros(W + 1, np.int64)
                np.cumsum(counts, out=id_start[1:])
                used = np.zeros(W, np.int64)
                slot0 = int(layout.win_slot_off[pos])
                grp0 = int(layout.win_grp_off[pos])
                s_off = 0
                gcells_list = cells[k][pos][1]
                for g, S in enumerate(layout.structure[pos]):
                    gcells = gcells_list[g] if g < len(gcells_list) else []
                    for p, (i, m) in enumerate(gcells):
                        prof[p, grp0 + g] = i
                        u = used[i]
                        toks = worder[id_start[i] + u : id_start[i] + u + m]
                        used[i] = u + m
                        dst_p[lo + toks] = p
                        dst_s[lo + toks] = slot0 + s_off + np.arange(m)
                    s_off += S
            fa[dst_p, dst_s, :] = fs
        in_maps.append(
            {
                "feats": fa.astype(ml_dtypes.bfloat16),
                "ids": prof,
                "iota": iota,
            }
        )
    return in_maps


# --------------------------------------------------------------------------
# device program
# --------------------------------------------------------------------------

def build_nc(layout):
    from concourse import bacc, mybir, tile

    bf16 = mybir.dt.bfloat16
    f32 = mybir.dt.float32
    nc = bacc.Bacc("TRN2", target_bir_lowering=False)
    feats_ext = nc.declare_dram_parameter(
        "feats", [128, layout.s_tot, C], bf16, isOutput=False
    )
    ids_ext = nc.declare_dram_parameter("ids", [128, layout.g_tot], f32, isOutput=False)
    iota_ext = nc.declare_dram_parameter("iota", [128, W], bf16, isOutput=False)
    n_chunks = N_WIN // CHUNK_W
    out_ext = nc.declare_dram_parameter("out", [128, N_WIN, C], bf16, isOutput=True)

    # chunk slot extents
    chunk_lo = [int(layout.win_slot_off[ch * CHUNK_W]) for ch in range(n_chunks)]
    chunk_hi = [int(layout.win_slot_off[(ch + 1) * CHUNK_W]) for ch in range(n_chunks)]
    max_chunk_slots = max(hi - lo for lo, hi in zip(chunk_lo, chunk_hi))

    with tile.TileContext(nc) as tc:
        with (
            tc.tile_pool(name="stage", bufs=3) as stagep,
            tc.tile_pool(name="oh", bufs=10) as ohp,
            tc.tile_pool(name="psum", bufs=8, space="PSUM") as psump,
            tc.tile_pool(name="fin", bufs=3) as finp,
            tc.tile_pool(name="cst", bufs=1) as cstp,
        ):
            iota_t = cstp.tile([128, W], bf16)
            nc.scalar.dma_start(out=iota_t[:], in_=iota_ext[:])
            ids_t = cstp.tile([128, layout.g_tot], f32)
            nc.scalar.dma_start(out=ids_t[:], in_=ids_ext[:])
            ot_all = cstp.tile([128, N_WIN, C], bf16)

            for ch in range(n_chunks):
                lo, hi = chunk_lo[ch], chunk_hi[ch]
                src = stagep.tile([128, max_chunk_slots, C], bf16, tag="src")
                nc.sync.dma_start(
                    out=src[:, : hi - lo, :], in_=feats_ext[:, lo:hi, :]
                )
                for wi in range(CHUNK_W):
                    w = ch * CHUNK_W + wi
                    sizes = layout.structure[w]
                    ps = psump.tile([128, C], f32, tag="ps")
                    n_slots = sum(sizes)
                    s_base = int(layout.win_slot_off[w]) - lo
                    g_base = int(layout.win_grp_off[w])
                    s_off = 0
                    for g, S in enumerate(sizes):
                        oh = ohp.tile([128, W], bf16, tag="oh")
                        nc.vector.tensor_scalar(
                            out=oh[:],
                            in0=iota_t[:],
                            scalar1=ids_t[:, g_base + g : g_base + g + 1],
                            scalar2=None,
                            op0=mybir.AluOpType.is_equal,
                        )
                        for s in range(S):
                            slot = s_base + s_off + s
                            nc.tensor.matmul(
                                out=ps[:],
                                lhsT=oh[:],
                                rhs=src[:, slot, :],
                                start=(s_off + s == 0),
                                stop=(s_off + s == n_slots - 1),
                            )
                        s_off += S
                    nc.scalar.activation(
                        ot_all[:, w, :], ps[:], mybir.ActivationFunctionType.Copy
                    )
                # output DMA in 5-chunk super-bursts on the idle Pool queue so
                # it can't head-of-line block input DMAs, with full-rate bursts
                if ch % 5 == 4 or ch == n_chunks - 1:
                    p0 = (ch - ch % 5) * CHUNK_W
                    p1 = (ch + 1) * CHUNK_W
                    nc.gpsimd.dma_start(
                        out=out_ext[:, p0:p1, :], in_=ot_all[:, p0:p1, :]
                    )
    nc.compile()
    return nc


def assemble_output(results, n_coarse, perm):
    out = np.empty((NCORES * RNG, C), np.float32)
    for k in range(NCORES):
        r = np.asarray(results[k]["out"], dtype=np.float32)  # [128, N_WIN, C]
        by_pos = r.transpose(1, 0, 2)
        core = out[k * RNG : (k + 1) * RNG].reshape(N_WIN, W, C)
        core[perm[k]] = by_pos
    return out[:n_coarse]


def _install_axon_hooks_shim():
    """Provide antenv.axon_hooks + the ctypes NTFF hook if the image lacks it."""
    import contextlib
    import ctypes
    import types

    try:
        from antenv.axon_hooks import get_axon_ntff_profile_hook  # noqa: F401

        return
    except ImportError:
        pass
    import antenv

    mod = types.ModuleType("antenv.axon_hooks")
    state = {"h": None}
    mod.set_axon_ntff_profile_hook = lambda h: state.__setitem__("h", h)
    mod.get_axon_ntff_profile_hook = lambda: state["h"]
    antenv.axon_hooks = mod
    sys.modules["antenv.axon_hooks"] = mod

    so_path = "/opt/axon/libaxon_pjrt.so"
    if not os.path.exists(so_path):
        return
    lib = ctypes.CDLL(so_path)
    if not hasattr(lib, "axon_start_nrt_profile"):
        return
    lib.axon_start_nrt_profile.argtypes = [
        ctypes.POINTER(ctypes.c_int64),
        ctypes.c_size_t,
    ]
    lib.axon_start_nrt_profile.restype = ctypes.c_int64
    lib.axon_stop_nrt_profile.argtypes = [ctypes.c_char_p]
    lib.axon_stop_nrt_profile.restype = ctypes.c_int64

    @contextlib.contextmanager
    def _hook(output_dir, device_ids):
        import jax

        jax.devices()
        if device_ids:
            idsv = (ctypes.c_int64 * len(device_ids))(*device_ids)
            rc = lib.axon_start_nrt_profile(idsv, len(device_ids))
        else:
            rc = lib.axon_start_nrt_profile(None, 0)
        if rc != 0:
            raise RuntimeError(f"axon_start_nrt_profile rc={rc}")
        try:
            yield
        finally:
            nfiles = lib.axon_stop_nrt_profile(str(output_dir).encode())
            print(f"profile: {nfiles} file(s) written to {output_dir}", file=sys.stderr)

    state["h"] = _hook


def _per_core_counts(coarse_ids):
    """[NCORES][N_WIN][W] token counts."""
    ids = np.asarray(coarse_ids, dtype=np.int64).ravel()
    cnt = np.bincount(ids, minlength=N_COARSE_PAD)
    return cnt.reshape(NCORES, N_WIN, W)


def kernel(fine_feats, coarse_ids, num_coarse):
    global LAST_RESULT
    from concourse.bass_utils import run_bass_kernel_spmd

    counts = _per_core_counts(coarse_ids)
    structure, cells, perm = build_structure(counts)
    layout = Layout(structure)
    in_maps = shard_inputs(fine_feats, coarse_ids, layout, cells, perm)
    key = tuple(tuple(s) for s in structure)
    if key not in _nc_cache:
        _nc_cache.clear()
        _nc_cache[key] = build_nc(layout)
    nc = _nc_cache[key]
    trace = bool(int(os.environ.get("KERNEL_TRACE", "0")))
    if trace:
        _install_axon_hooks_shim()
    res = run_bass_kernel_spmd(nc, in_maps, core_ids=list(range(NCORES)), trace=trace)
    LAST_RESULT = res
    return assemble_output(res.results, int(num_coarse), perm)


# revision 30
# speedup vs baseline: 1.0884x; 1.0775x over previous
"""Sparse avg-pool (segment mean) for Trainium2, 8 NeuronCores — grouped-profile version.

Range-shard coarse ids across cores (core k owns ids [k*31360, (k+1)*31360)),
so no collective is needed.  Each core's 31360 ids form 245 windows of 128
consecutive ids.  The segment-sum runs on the TensorEngine as
onehot^T @ feats accumulated per window in PSUM.

The onehot build is the DVE bottleneck (~163-195ns per tensor_scalar with an
AP scalar, regardless of free size), so the packing amortizes one onehot over
as many matmul slots as possible: a *group* is a set of S slots sharing one
id-profile prof[p] — partition p of every slot in the group holds tokens of
window-relative id prof[p].  The host decomposes each window's id counts into
groups from a size menu ((5,4)/(6,4)/(6,5)... for 2-group windows,
(4,3,1)/(5,3,1)... for 3-group ones), balancing DVE op count against input
DMA bytes (both sit near the wall); half the window positions use the denser
3-group menu, interleaved so every chunk stays locally balanced.  One
`tensor_scalar is_equal` per group (iota row vs per-partition f32 id —
unit-stride bf16 operands, 4x DVE mode) builds the onehot for all S matmuls
of the group.

The group structure is data-dependent and baked into the program at build
time; windows are aligned across cores by heaviness rank and the per-position
sizes unioned, so a single SPMD program serves all 8 cores (a core's cell
assignment stays valid under any elementwise-larger size vector).  Features
are pre-scaled by 1/count on the host (exact int bincount), so PSUM
accumulates the mean directly; the ACT engine copies PSUM out as bf16 to
halve the output DMA, and output DMAs issue from the idle GPSIMD queue so
they can't head-of-line block the next chunk's input DMA on the Sync queue
(that stall was worth ~80us of TensorE idle + pstate resets).
"""
import os
import sys

sys.path.insert(0, "/opt/trn_rl_repo")

import numpy as np

NCORES = 8
C = 64
W = 128          # ids per window
N_WIN = 245      # windows per core
RNG = N_WIN * W  # ids per core (31360)
N_COARSE_PAD = NCORES * RNG
CHUNK_W = 7      # windows per DMA chunk (245 = 35 * 7)

_nc_cache = {}
LAST_RESULT = None


# --------------------------------------------------------------------------
# host-side packing
# --------------------------------------------------------------------------

def _reduce_to_budget(x, l, A, B):
    """Convert A-cells to leftover until sum(x) <= 128, preferring ids where
    the extra A tokens add the fewest B-cells.  Mutates x, l; returns ok."""
    over = int(x.sum()) - 128
    while over > 0:
        cand = np.nonzero(x > 0)[0]
        if cand.size == 0:
            return False
        delta = (-(-(l[cand] + A) // B)) - (-(-l[cand] // B))
        i = int(cand[np.argmin(delta)])
        x[i] -= 1
        l[i] += A
        over -= 1
    return True


def _emit_cells(per_id, S):
    """per_id[i] = token count to place in <=S-token cells of id i."""
    out = []
    for i in np.nonzero(per_id)[0]:
        ci = int(per_id[i])
        while ci > 0:
            m = min(S, ci)
            out.append((int(i), m))
            ci -= m
    return out


def _fit_sizes(c, sizes):
    """Decompose counts c into groups of the given sizes (desc).  Returns
    cells-per-group or None."""
    l = c.astype(np.int64).copy()
    groups = []
    for j, S in enumerate(sizes):
        nxt = sizes[j + 1] if j + 1 < len(sizes) else None
        x = l // S
        rem = l - S * x
        if nxt is None:
            if int((-(-l // S)).sum()) > 128:
                return None
            groups.append(_emit_cells(l, S))
            l = np.zeros_like(l)
        else:
            if not _reduce_to_budget(x, rem, S, nxt):
                return None
            groups.append(_emit_cells(x * S, S))
            l = rem
    if int(l.sum()) != 0:
        return None
    return groups


MENU2 = [(5, 4), (6, 4), (6, 5), (6, 5, 2), (6, 5, 5)]
MENU3 = [(4, 3, 1), (5, 3, 1), (5, 4, 1), (5, 4, 2), (6, 5, 2), (6, 5, 5)]
DENSE_FRACTION = 0.6  # fraction of window positions packed with the 3-group menu


def _cells_5x(c):
    """Fallback: (5,5,...) decomposition, biggest cells first."""
    cells = _emit_cells(c, 5)
    cells.sort(key=lambda t: -t[1])
    groups = [cells[j : j + 128] for j in range(0, len(cells), 128)]
    if not groups:
        groups = [[]]
    sizes = [max((m for _, m in g), default=1) for g in groups]
    return sizes, groups, 99

def _decomp_menu(counts, menu):
    for rank, sizes in enumerate(menu):
        g = _fit_sizes(counts, list(sizes))
        if g is not None:
            return list(sizes), g, rank
    return _cells_5x(counts)


def build_structure(per_core_counts):
    """Per-core decompositions, heaviness-sorted window alignment, union sizes.

    Returns (structure, cells, perm):
      structure[w] = union group sizes at aligned position w
      cells[k][w]  = cell lists for core k's window at position w
      perm[k][w]   = the actual window index of core k at position w
    """
    cells = [[None] * N_WIN for _ in range(NCORES)]
    perm = np.zeros((NCORES, N_WIN), np.int64)
    n_dense = int(N_WIN * DENSE_FRACTION)
    for k in range(NCORES):
        rows = []
        for w in range(N_WIN):
            c = per_core_counts[k][w]
            sizes_k, cells_k, rank = _decomp_menu(c, MENU2)
            rows.append((rank, -int(c.sum()), w, c))
        rows.sort()
        for pos, (_, _, w, c) in enumerate(rows):
            # densest positions use the 3-group menu (lower slot count, one
            # extra DVE op); positions are aligned across cores so the
            # menu split must be position-based, not per-core
            dense = int((pos + 1) * DENSE_FRACTION) > int(pos * DENSE_FRACTION)
            menu = MENU3 if dense else MENU2
            sizes_k, cells_k, _ = _decomp_menu(c, menu)
            perm[k][pos] = w
            cells[k][pos] = (sizes_k, cells_k)
    structure = []
    for pos in range(N_WIN):
        union = []
        for k in range(NCORES):
            for j, s in enumerate(cells[k][pos][0]):
                if j < len(union):
                    union[j] = max(union[j], s)
                else:
                    union.append(s)
        if not union:
            union = [1]
        structure.append(union)
    return structure, cells, perm


class Layout:
    """Slot/group offsets derived from the union structure."""

    def __init__(self, structure):
        self.structure = structure
        self.win_slot_off = np.zeros(N_WIN + 1, np.int64)
        self.win_grp_off = np.zeros(N_WIN + 1, np.int64)
        for w, sizes in enumerate(structure):
            self.win_slot_off[w + 1] = self.win_slot_off[w] + sum(sizes)
            self.win_grp_off[w + 1] = self.win_grp_off[w] + len(sizes)
        self.s_tot = int(self.win_slot_off[-1])
        self.g_tot = int(self.win_grp_off[-1])


def shard_inputs(feats, ids, layout, cells, perm):
    """Route rows to owner cores and place tokens per the precomputed cells.

    Builds, per core, index arrays (partition, slot) for every token and does
    one vectorized scatter of the pre-scaled features.
    """
    import ml_dtypes

    ids = np.asarray(ids, dtype=np.int64).ravel()
    feats = np.asarray(feats, dtype=np.float32)

    cnt = np.bincount(ids, minlength=N_COARSE_PAD).astype(np.float32)
    scale = 1.0 / np.maximum(cnt, 1.0)
    feats_scaled = feats * scale[ids][:, None]

    owner = ids // RNG
    order = np.argsort(owner, kind="stable")
    counts_per_core = np.bincount(owner, minlength=NCORES)
    offs = np.zeros(NCORES + 1, np.int64)
    np.cumsum(counts_per_core, out=offs[1:])
    feats_sorted = feats_scaled[order]
    local_sorted = (ids - owner * RNG)[order]

    iota = np.broadcast_to(
        np.arange(W, dtype=np.float32), (128, W)
    ).astype(ml_dtypes.bfloat16)

    in_maps = []
    for k in range(NCORES):
        fk = feats_sorted[offs[k] : offs[k + 1]]
        lk = local_sorted[offs[k] : offs[k + 1]]
        fa = np.zeros((128, layout.s_tot, C), np.float32)
        prof = np.zeros((128, layout.g_tot), np.float32)
        if lk.shape[0]:
            sorder = np.argsort(lk, kind="stable")
            ls = lk[sorder]
            fs = fk[sorder]
            win = ls >> 7
            wstart = np.searchsorted(win, np.arange(N_WIN + 1))
            dst_p = np.empty(ls.shape[0], np.int64)
            dst_s = np.empty(ls.shape[0], np.int64)
            for pos in range(N_WIN):
                w = int(perm[k][pos])
                lo, hi = int(wstart[w]), int(wstart[w + 1])
                if lo == hi:
                    continue
                wrel = (ls[lo:hi] & 127).astype(np.int64)
                worder = np.argsort(wrel, kind="stable")
                counts = np.bincount(wrel, minlength=W)
                id_start = np.zeros(W + 1, np.int64)
                np.cumsum(counts, out=id_start[1:])
                used = np.zeros(W, np.int64)
                slot0 = int(layout.win_slot_off[pos])
                grp0 = int(layout.win_grp_off[pos])
                s_off = 0
                gcells_list = cells[k][pos][1]
                for g, S in enumerate(layout.structure[pos]):
                    gcells = gcells_list[g] if g < len(gcells_list) else []
                    for p, (i, m) in enumerate(gcells):
                        prof[p, grp0 + g] = i
                        u = used[i]
                        toks = worder[id_start[i] + u : id_start[i] + u + m]
                        used[i] = u + m
                        dst_p[lo + toks] = p
                        dst_s[lo + toks] = slot0 + s_off + np.arange(m)
                    s_off += S
            fa[dst_p, dst_s, :] = fs
        in_maps.append(
            {
                "feats": fa.astype(ml_dtypes.bfloat16),
                "ids": prof,
                "iota": iota,
            }
        )
    return in_maps


# --------------------------------------------------------------------------
# device program
# --------------------------------------------------------------------------

def build_nc(layout):
    from concourse import bacc, mybir, tile

    bf16 = mybir.dt.bfloat16
    f32 = mybir.dt.float32
    nc = bacc.Bacc("TRN2", target_bir_lowering=False)
    feats_ext = nc.declare_dram_parameter(
        "feats", [128, layout.s_tot, C], bf16, isOutput=False
    )
    ids_ext = nc.declare_dram_parameter("ids", [128, layout.g_tot], f32, isOutput=False)
    iota_ext = nc.declare_dram_parameter("iota", [128, W], bf16, isOutput=False)
    n_chunks = N_WIN // CHUNK_W
    out_ext = nc.declare_dram_parameter("out", [128, N_WIN, C], bf16, isOutput=True)

    # chunk slot extents
    chunk_lo = [int(layout.win_slot_off[ch * CHUNK_W]) for ch in range(n_chunks)]
    chunk_hi = [int(layout.win_slot_off[(ch + 1) * CHUNK_W]) for ch in range(n_chunks)]
    max_chunk_slots = max(hi - lo for lo, hi in zip(chunk_lo, chunk_hi))

    with tile.TileContext(nc) as tc:
        with (
            tc.tile_pool(name="stage", bufs=3) as stagep,
            tc.tile_pool(name="oh", bufs=10) as ohp,
            tc.tile_pool(name="psum", bufs=8, space="PSUM") as psump,
            tc.tile_pool(name="fin", bufs=3) as finp,
            tc.tile_pool(name="cst", bufs=1) as cstp,
        ):
            iota_t = cstp.tile([128, W], bf16)
            nc.scalar.dma_start(out=iota_t[:], in_=iota_ext[:])
            ids_t = cstp.tile([128, layout.g_tot], f32)
            nc.scalar.dma_start(out=ids_t[:], in_=ids_ext[:])
            ot_all = cstp.tile([128, N_WIN, C], bf16)

            for ch in range(n_chunks):
                lo, hi = chunk_lo[ch], chunk_hi[ch]
                src = stagep.tile([128, max_chunk_slots, C], bf16, tag="src")
                nc.sync.dma_start(
                    out=src[:, : hi - lo, :], in_=feats_ext[:, lo:hi, :]
                )
                for wi in range(CHUNK_W):
                    w = ch * CHUNK_W + wi
                    sizes = layout.structure[w]
                    ps = psump.tile([128, C], f32, tag="ps")
                    n_slots = sum(sizes)
                    s_base = int(layout.win_slot_off[w]) - lo
                    g_base = int(layout.win_grp_off[w])
                    s_off = 0
                    for g, S in enumerate(sizes):
                        oh = ohp.tile([128, W], bf16, tag="oh")
                        if g == 2 and wi % 2 == 0:
                            # offload some third-group onehots to the
                            # half-idle ACT engine: relu(1 - (id - iota)^2)
                            # is exactly the onehot on the integer grid
                            sq = ohp.tile([128, W], f32, tag="sq")
                            nc.scalar.activation(
                                sq[:], iota_t[:],
                                mybir.ActivationFunctionType.Square,
                                bias=ids_t[:, g_base + g : g_base + g + 1],
                                scale=-1.0,
                            )
                            nc.scalar.activation(
                                oh[:], sq[:],
                                mybir.ActivationFunctionType.Relu,
                                bias=1.0, scale=-1.0,
                            )
                        else:
                            nc.vector.tensor_scalar(
                                out=oh[:],
                                in0=iota_t[:],
                                scalar1=ids_t[:, g_base + g : g_base + g + 1],
                                scalar2=None,
                                op0=mybir.AluOpType.is_equal,
                            )
                        for s in range(S):
                            slot = s_base + s_off + s
                            nc.tensor.matmul(
                                out=ps[:],
                                lhsT=oh[:],
                                rhs=src[:, slot, :],
                                start=(s_off + s == 0),
                                stop=(s_off + s == n_slots - 1),
                            )
                        s_off += S
                    nc.scalar.activation(
                        ot_all[:, w, :], ps[:], mybir.ActivationFunctionType.Copy
                    )
                # output DMA in 5-chunk super-bursts on the idle Pool queue so
                # it can't head-of-line block input DMAs, with full-rate
                # bursts; per-chunk in the last super-chunk to shrink the tail
                if ch % 5 == 4 or ch >= n_chunks - n_chunks % 5:
                    p0 = (ch - ch % 5) * CHUNK_W if ch % 5 == 4 else ch * CHUNK_W
                    p1 = (ch + 1) * CHUNK_W
                    nc.gpsimd.dma_start(
                        out=out_ext[:, p0:p1, :], in_=ot_all[:, p0:p1, :]
                    )
    nc.compile()
    return nc


def assemble_output(results, n_coarse, perm):
    out = np.empty((NCORES * RNG, C), np.float32)
    for k in range(NCORES):
        r = np.asarray(results[k]["out"], dtype=np.float32)  # [128, N_WIN, C]
        by_pos = r.transpose(1, 0, 2)
        core = out[k * RNG : (k + 1) * RNG].reshape(N_WIN, W, C)
        core[perm[k]] = by_pos
    return out[:n_coarse]


def _install_axon_hooks_shim():
    """Provide antenv.axon_hooks + the ctypes NTFF hook if the image lacks it."""
    import contextlib
    import ctypes
    import types

    try:
        from antenv.axon_hooks import get_axon_ntff_profile_hook  # noqa: F401

        return
    except ImportError:
        pass
    import antenv

    mod = types.ModuleType("antenv.axon_hooks")
    state = {"h": None}
    mod.set_axon_ntff_profile_hook = lambda h: state.__setitem__("h", h)
    mod.get_axon_ntff_profile_hook = lambda: state["h"]
    antenv.axon_hooks = mod
    sys.modules["antenv.axon_hooks"] = mod

    so_path = "/opt/axon/libaxon_pjrt.so"
    if not os.path.exists(so_path):
        return
    lib = ctypes.CDLL(so_path)
    if not hasattr(lib, "axon_start_nrt_profile"):
        return
    lib.axon_start_nrt_profile.argtypes = [
        ctypes.POINTER(ctypes.c_int64),
        ctypes.c_size_t,
    ]
    lib.axon_start_nrt_profile.restype = ctypes.c_int64
    lib.axon_stop_nrt_profile.argtypes = [ctypes.c_char_p]
    lib.axon_stop_nrt_profile.restype = ctypes.c_int64

    @contextlib.contextmanager
    def _hook(output_dir, device_ids):
        import jax

        jax.devices()
        if device_ids:
            idsv = (ctypes.c_int64 * len(device_ids))(*device_ids)
            rc = lib.axon_start_nrt_profile(idsv, len(device_ids))
        else:
            rc = lib.axon_start_nrt_profile(None, 0)
        if rc != 0:
            raise RuntimeError(f"axon_start_nrt_profile rc={rc}")
        try:
            yield
        finally:
            nfiles = lib.axon_stop_nrt_profile(str(output_dir).encode())
            print(f"profile: {nfiles} file(s) written to {output_dir}", file=sys.stderr)

    state["h"] = _hook


def _per_core_counts(coarse_ids):
    """[NCORES][N_WIN][W] token counts."""
    ids = np.asarray(coarse_ids, dtype=np.int64).ravel()
    cnt = np.bincount(ids, minlength=N_COARSE_PAD)
    return cnt.reshape(NCORES, N_WIN, W)


def kernel(fine_feats, coarse_ids, num_coarse):
    global LAST_RESULT
    from concourse.bass_utils import run_bass_kernel_spmd

    counts = _per_core_counts(coarse_ids)
    structure, cells, perm = build_structure(counts)
    layout = Layout(structure)
    in_maps = shard_inputs(fine_feats, coarse_ids, layout, cells, perm)
    key = tuple(tuple(s) for s in structure)
    if key not in _nc_cache:
        _nc_cache.clear()
        _nc_cache[key] = build_nc(layout)
    nc = _nc_cache[key]
    trace = bool(int(os.environ.get("KERNEL_TRACE", "0")))
    if trace:
        _install_axon_hooks_shim()
    res = run_bass_kernel_spmd(nc, in_maps, core_ids=list(range(NCORES)), trace=trace)
    LAST_RESULT = res
    return assemble_output(res.results, int(num_coarse), perm)
